# revision 1
# baseline (speedup 1.0000x reference)
"""VQ codebook (N=524288, D=384, K=512) on 8 TRN2 NeuronCores.

Data-parallel: z sharded along N (65536 rows/core), codebook replicated.
Per core: normalize W once; per 128-row tile: PE transpose z -> f32r
scores matmul s = z @ Wn^T -> DVE max/max_index on PSUM -> SWDGE gather
Wn[idx] -> DMA out. Loss reconstructed from per-row scalars
(d_min = znsq + 1 - 2*rnorm*smax), final mean on host.

kernel(z, W) -> (z_q_st [N,384] f32, idx [N] i32, loss scalar f32)
"""

import sys

sys.path.insert(0, "/opt/trn_rl_repo")

import numpy as np

import concourse.bacc as bacc
import concourse.bass as bass
import concourse.mybir as mybir
import concourse.tile as tile
from concourse.masks import make_identity
from concourse.tile_rust import add_dep_helper
from concourse.bass_utils import run_bass_kernel_spmd

F32 = mybir.dt.float32
F32R = mybir.dt.float32r
U32 = mybir.dt.uint32

P = 128
D = 384
K = 512
DC = D // P
KA = K // P
N_CORES = 8
N_TOTAL = 524288
N_SHARD = N_TOTAL // N_CORES

Alu = mybir.AluOpType
Act = mybir.ActivationFunctionType


def build_kernel(n_rows: int, use_f32r: bool = True):
    assert n_rows % P == 0
    T = n_rows // P
    nc = bacc.Bacc("TRN2", target_bir_lowering=False, debug=False)

    z = nc.dram_tensor("z", [n_rows, D], F32, kind="ExternalInput").ap()
    w = nc.dram_tensor("w", [K, D], F32, kind="ExternalInput").ap()
    zq = nc.dram_tensor("zq", [n_rows, D], F32, kind="ExternalOutput").ap()
    idxt = nc.dram_tensor("idxt", [P, T], U32, kind="ExternalOutput").ap()
    lossv = nc.dram_tensor("lossv", [P, 1], F32, kind="ExternalOutput").ap()
    wn_table = nc.dram_tensor("wn_table", [K, D], F32).ap()

    MMDT = F32R if use_f32r else F32

    with tile.TileContext(nc) as tc:
        with (
            tc.tile_pool(name="constp", bufs=1) as constp,
            tc.tile_pool(name="accp", bufs=1) as accp,
            tc.tile_pool(name="iop", bufs=4) as iop,
            tc.tile_pool(name="pstp", bufs=2, space="PSUM") as pstp,
            tc.tile_pool(name="pssp", bufs=2, space="PSUM") as pssp,
        ):
            ident = constp.tile([P, P], F32)
            make_identity(nc, ident[:])

            wrows = constp.tile([P, KA * D], F32)
            nc.sync.dma_start(
                out=wrows[:].rearrange("p (a d) -> p a d", a=KA),
                in_=w.rearrange("(a p) d -> p a d", p=P),
            )

            wss = constp.tile([P, KA], F32)
            scr = constp.tile([P, D], F32)
            for a in range(KA):
                nc.scalar.activation(
                    out=scr[:],
                    in_=wrows[:, a * D : (a + 1) * D],
                    func=Act.Square,
                    accum_out=wss[:, a : a + 1],
                )
            wnorm = constp.tile([P, KA], F32)
            nc.scalar.sqrt(wnorm[:], wss[:])
            wrnorm = constp.tile([P, KA], F32)
            nc.vector.reciprocal(wrnorm[:], wnorm[:])

            wn = constp.tile([P, KA * D], F32)
            wn_muls = []
            for a in range(KA):
                wn_muls.append(
                    nc.vector.tensor_scalar_mul(
                        wn[:, a * D : (a + 1) * D],
                        wrows[:, a * D : (a + 1) * D],
                        wrnorm[:, a : a + 1],
                    )
                )

            nc.sync.dma_start(
                out=wn_table.rearrange("(a p) d -> p a d", p=P),
                in_=wn[:].rearrange("p (a d) -> p a d", a=KA),
            )

            def pe_sync(deps, hint):
                n = nc.tensor.nop(nofuse=True, hint=hint)
                for d in deps:
                    if d is not None:
                        add_dep_helper(n.ins, d.ins, reason="pe_sync")
                return n

            wnt = constp.tile([P, DC * K], MMDT)
            blk_copies = []
            for a in range(KA):
                for c in range(DC):
                    i = a * DC + c
                    pe_sync(
                        [wn_muls[a], blk_copies[i - 2] if i >= 2 else None],
                        hint=f"w_tr_{i}",
                    )
                    blk_ps = pstp.tile([P, P], F32, space="PSUM", tag="blk")
                    nc.tensor.transpose(
                        out=blk_ps[:],
                        in_=wn[:, a * D + c * P : a * D + (c + 1) * P],
                        identity=ident[:],
                    )
                    blk_copies.append(
                        nc.scalar.copy(
                            wnt[:, c * K + a * P : c * K + (a + 1) * P],
                            blk_ps[:],
                        )
                    )

            zss = accp.tile([P, T], F32)
            smax8 = accp.tile([P, 8 * T], F32)
            idxall = accp.tile([P, 8 * T], U32)

            ztcopy_hist = []
            mi_hist = []
            for t in range(T):
                zrow = iop.tile([P, D], F32, tag="zrow")
                zdma = nc.sync.dma_start(
                    out=zrow[:], in_=z[t * P : (t + 1) * P, :]
                )

                zsq_scr = iop.tile([P, D], F32, tag="zsq_scr")
                nc.scalar.activation(
                    out=zsq_scr[:],
                    in_=zrow[:],
                    func=Act.Square,
                    accum_out=zss[:, t : t + 1],
                )

                pe_sync(
                    [zdma, ztcopy_hist[t - 2] if t >= 2 else blk_copies[-1]],
                    hint=f"tr_{t}",
                )
                ztp = pstp.tile([P, D], F32, space="PSUM", tag="ztp")
                for c in range(DC):
                    nc.tensor.transpose(
                        out=ztp[:, c * P : (c + 1) * P],
                        in_=zrow[:, c * P : (c + 1) * P],
                        identity=ident[:],
                    )
                zt = iop.tile([P, D], MMDT, tag="zt")
                ztcopy_hist.append(nc.scalar.copy(zt[:], ztp[:]))

                pe_sync(
                    [ztcopy_hist[t], mi_hist[t - 2] if t >= 2 else None],
                    hint=f"mm_{t}",
                )
                sps = pssp.tile([P, K], F32, space="PSUM", tag="sps")
                for c in range(DC):
                    nc.tensor.matmul(
                        out=sps[:],
                        lhsT=zt[:, c * P : (c + 1) * P],
                        rhs=wnt[:, c * K : (c + 1) * K],
                        start=(c == 0),
                        stop=(c == DC - 1),
                    )

                nc.vector.max(smax8[:, 8 * t : 8 * t + 8], sps[:])
                mi_hist.append(
                    nc.vector.max_index(
                        out=idxall[:, 8 * t : 8 * t + 8],
                        in_max=smax8[:, 8 * t : 8 * t + 8],
                        in_values=sps[:],
                    )
                )

                g = iop.tile([P, D], F32, tag="g")
                nc.gpsimd.indirect_dma_start(
                    out=g[:],
                    out_offset=None,
                    in_=wn_table[:],
                    in_offset=bass.IndirectOffsetOnAxis(
                        ap=idxall[:, 8 * t : 8 * t + 1], axis=0
                    ),
                )
                nc.sync.dma_start(out=zq[t * P : (t + 1) * P, :], in_=g[:])

            # loss tail: d_min = znsq + 1 - 2*rnorm*smax, summed per partition
            rsq = accp.tile([P, T], F32)
            nc.vector.reciprocal(rsq[:], zss[:])
            rn = accp.tile([P, T], F32)
            nc.scalar.sqrt(rn[:], rsq[:])
            znsq = accp.tile([P, T], F32)
            nc.vector.tensor_tensor(
                out=znsq[:], in0=zss[:], in1=rsq[:], op=Alu.mult
            )
            smax = accp.tile([P, T], F32)
            nc.vector.tensor_copy(
                out=smax[:],
                in_=smax8[:].rearrange("p (t e) -> p t e", e=8)[:, :, 0],
            )
            rs = accp.tile([P, T], F32)
            nc.vector.tensor_tensor(out=rs[:], in0=rn[:], in1=smax[:], op=Alu.mult)
            r1 = accp.tile([P, 1], F32)
            nc.vector.tensor_reduce(
                out=r1[:], in_=rs[:], axis=mybir.AxisListType.X, op=Alu.add
            )
            r2 = accp.tile([P, 1], F32)
            nc.vector.tensor_reduce(
                out=r2[:], in_=znsq[:], axis=mybir.AxisListType.X, op=Alu.add
            )
            r3 = accp.tile([P, 1], F32)
            nc.vector.tensor_scalar(
                out=r3[:], in0=r1[:], scalar1=-2.0, scalar2=None, op0=Alu.mult
            )
            r4 = accp.tile([P, 1], F32)
            nc.vector.tensor_tensor(out=r4[:], in0=r3[:], in1=r2[:], op=Alu.add)
            r5 = accp.tile([P, 1], F32)
            nc.vector.tensor_scalar_add(r5[:], r4[:], float(T))
            nc.sync.dma_start(out=lossv[:, :], in_=r5[:])

            idxc = accp.tile([P, T], U32)
            nc.vector.tensor_copy(
                out=idxc[:],
                in_=idxall[:].rearrange("p (t e) -> p t e", e=8)[:, :, 0],
            )
            nc.sync.dma_start(out=idxt[:, :], in_=idxc[:])

    nc.compile()
    return nc


_NC_CACHE = {}


def _get_nc():
    if "nc" not in _NC_CACHE:
        _NC_CACHE["nc"] = build_kernel(N_SHARD, use_f32r=True)
    return _NC_CACHE["nc"]


def kernel(z: np.ndarray, W: np.ndarray):
    z = np.ascontiguousarray(np.asarray(z, dtype=np.float32))
    W = np.ascontiguousarray(np.asarray(W, dtype=np.float32))
    assert z.shape == (N_TOTAL, D) and W.shape == (K, D)
    nc = _get_nc()

    in_maps = [
        {"z": z[c * N_SHARD : (c + 1) * N_SHARD], "w": W}
        for c in range(N_CORES)
    ]
    res = run_bass_kernel_spmd(nc, in_maps, list(range(N_CORES)))

    z_q = np.concatenate(
        [res.results[c]["zq"] for c in range(N_CORES)], axis=0
    )
    idx = np.concatenate(
        [res.results[c]["idxt"].T.reshape(-1) for c in range(N_CORES)]
    ).astype(np.int32)
    total = sum(
        res.results[c]["lossv"].sum(dtype=np.float64) for c in range(N_CORES)
    )
    loss = np.float32(1.25 * total / (N_TOTAL * D))
    return z_q, idx, loss
